# revision 1
# baseline (speedup 1.0000x reference)
"""Linear attention (non-causal, elu+1 feature map) on 8 Trainium2 cores.

Math per (batch b, head h), with phi(x) = elu(x)+1:
    C_aug = phi(K)^T @ [V | 1]        # (64, 65): context (64x64) + k_sum col
    numer = phi(Q) @ C_aug[:, :64]    # (T, 64)
    denom = phi(Q) @ C_aug[:, 64]     # (T,)
    out   = numer / denom             # eps=1e-6 is negligible vs denom ~1e5

Sharding: 16 heads / 8 cores = 2 heads per core, all 4 batches per core
(per-head problems are fully independent). Host pre-transposes Q per core
to (e, t) layout so every device matmul contracts along SBUF partitions
with zero on-device transposes, packs [K | V | 1] per head into one
tensor (one DMA per head -> one DMA-queue wait per matmul; the ISA allows
only 2 sync waits on a weight-load), and the ones column makes k_sum fall
out of matmul1 for free.

Device layouts (per core, all f32, all DMA-contiguous):
    qt:  (4, 128, 4096)    qt[b, hh*64+e, t] = Q[b, t, head(hh)*64+e]
    kva: (4, 2, 4096, 129) [K | V | 1] per head
    o:   (4, 2, 4096, 64)  natural per-head output

t-blocking: kva tiles assign t = p*32 + n (partition p, tile n) so each
DMA reads ~16KB contiguous per partition; matmul2 uses strided lhsT
column chunks (t = j*32 + n) so output blocks land contiguous in HBM too.
The t->(partition, tile) assignment is sum-invariant for matmul1 and
self-consistent for matmul2's output indexing.
"""

from contextlib import ExitStack

import numpy as np

import concourse.bacc as bacc
import concourse.bass as bass
import concourse.mybir as mybir
import concourse.tile as tile
from concourse.bass_utils import run_bass_kernel_spmd

B = 4
T = 4096
D = 1024
H = 16
E = 64
EA = E + 1
W = E + EA  # 129 cols per kva row
NCORES = 8
HPC = H // NCORES  # 2 heads per core
P = 128
NT = T // P  # 32 t-tiles
GRP = 4  # matmul2 chunks per psum group
DT = mybir.dt.float32
AF = mybir.ActivationFunctionType
ALU = mybir.AluOpType


def _phi(nc, x, tmp):
    """x <- elu(x)+1 == max(x+1, exp(min(x, 0))), tmp as scratch.

    x and tmp may be multi-dim APs of matching shape.
    """
    nc.vector.tensor_scalar_min(tmp, x, 0.0)
    nc.scalar.activation(tmp, tmp, AF.Exp)
    nc.vector.scalar_tensor_tensor(x, x, 1.0, tmp, ALU.add, ALU.max)


def build_nc():
    nc = bacc.Bacc("TRN2", target_bir_lowering=False, debug=False)
    qt = nc.dram_tensor("qt", [B, P, T], DT, kind="ExternalInput").ap()
    kva = nc.dram_tensor("kva", [B, HPC, T, W], DT, kind="ExternalInput").ap()
    o = nc.dram_tensor("o", [B, HPC, T, E], DT, kind="ExternalOutput").ap()

    with tile.TileContext(nc) as tc, ExitStack() as ctx:
        qt_pool = ctx.enter_context(tc.tile_pool(name="qt", bufs=2))
        mq_pool = ctx.enter_context(tc.tile_pool(name="mq", bufs=2))
        kva_pool = ctx.enter_context(tc.tile_pool(name="kva", bufs=3))
        mk_pool = ctx.enter_context(tc.tile_pool(name="mk", bufs=2))
        c_pool = ctx.enter_context(tc.tile_pool(name="c", bufs=2))
        r_pool = ctx.enter_context(tc.tile_pool(name="r", bufs=4))
        out_pool = ctx.enter_context(tc.tile_pool(name="out", bufs=2))
        psc_pool = ctx.enter_context(tc.tile_pool(name="psc", bufs=2, space="PSUM"))
        pso_pool = ctx.enter_context(tc.tile_pool(name="pso", bufs=4, space="PSUM"))

        for b in range(B):
            # Q^T for both heads: (128, 4096), partition = hh*64+e
            qt_t = qt_pool.tile([P, T], DT)
            nc.sync.dma_start(qt_t[:], qt[b])
            mq = mq_pool.tile([P, T], DT)
            _phi(nc, qt_t[:], mq[:])

            # ---- matmul1: C_aug[e, m] = sum_t phiK[t, e] * [V|1][t, m] ----
            # head 0 -> psum partitions 0:64, head 1 -> 64:128 (col tiling)
            psum_c = psc_pool.tile([P, EA], DT)
            for h in range(HPC):
                kva_t = kva_pool.tile([P, NT * W], DT)
                nc.sync.dma_start(
                    kva_t[:].rearrange("p (n e) -> p n e", e=W),
                    kva[b, h].rearrange("(p n) e -> p n e", p=P),
                )
                k3 = kva_t[:].rearrange("p (n e) -> p n e", e=W)[:, :, 0:E]
                mk = mk_pool.tile([P, NT * E], DT)
                _phi(nc, k3, mk[:].rearrange("p (n e) -> p n e", e=E))
                for n in range(NT):
                    nc.tensor.matmul(
                        psum_c[h * E : (h + 1) * E, :],
                        lhsT=kva_t[:, n * W : n * W + E],
                        rhs=kva_t[:, n * W + E : (n + 1) * W],
                        start=(n == 0),
                        stop=(n == NT - 1),
                        tile_position=(0, h * E),
                    )
            c_sb = c_pool.tile([P, EA], DT)
            nc.vector.tensor_copy(c_sb[:], psum_c[:])

            # ---- matmul2 + normalize: out[t, d] = phiQ[t,:] @ C[:, d] / denom[t]
            for h in range(HPC):
                out_sb = out_pool.tile([P, NT * E], DT)
                for g in range(NT // GRP):
                    ps_o = pso_pool.tile([P, GRP * EA], DT)
                    for j in range(GRP):
                        n = g * GRP + j
                        nc.tensor.matmul(
                            ps_o[:, j * EA : (j + 1) * EA],
                            lhsT=qt_t[h * E : (h + 1) * E, n::NT],
                            rhs=c_sb[h * E : (h + 1) * E, :],
                            start=True,
                            stop=True,
                        )
                    r_sb = r_pool.tile([P, GRP], DT)
                    nc.vector.reciprocal(r_sb[:], ps_o[:, E::EA])
                    for j in range(GRP):
                        n = g * GRP + j
                        nc.vector.tensor_scalar_mul(
                            out_sb[:, n * E : (n + 1) * E],
                            ps_o[:, j * EA : j * EA + E],
                            r_sb[:, j : j + 1],
                        )
                nc.sync.dma_start(
                    o[b, h].rearrange("(p n) e -> p n e", p=P),
                    out_sb[:].rearrange("p (n e) -> p n e", e=E),
                )
    nc.finalize()
    return nc


_NC_CACHE = None


def _get_nc():
    global _NC_CACHE
    if _NC_CACHE is None:
        _NC_CACHE = build_nc()
    return _NC_CACHE


def make_in_maps(query, key, value):
    query = np.ascontiguousarray(query, dtype=np.float32)
    key = np.ascontiguousarray(key, dtype=np.float32)
    value = np.ascontiguousarray(value, dtype=np.float32)
    in_maps = []
    for c in range(NCORES):
        lo = c * HPC * E
        hi = lo + HPC * E
        qt = np.ascontiguousarray(query[:, :, lo:hi].transpose(0, 2, 1))
        kva = np.empty((B, HPC, T, W), np.float32)
        kva[..., :E] = key[:, :, lo:hi].reshape(B, T, HPC, E).transpose(0, 2, 1, 3)
        kva[..., E : E + E] = (
            value[:, :, lo:hi].reshape(B, T, HPC, E).transpose(0, 2, 1, 3)
        )
        kva[..., E + E] = 1.0
        in_maps.append({"qt": qt, "kva": kva})
    return in_maps


def assemble_out(results):
    out = np.empty((B, T, D), np.float32)
    for c in range(NCORES):
        oc = results[c]["o"]  # (B, HPC, T, E)
        out[:, :, c * HPC * E : (c + 1) * HPC * E] = oc.transpose(0, 2, 1, 3).reshape(
            B, T, HPC * E
        )
    return out


def run(query, key, value, **spmd_kwargs):
    nc = _get_nc()
    in_maps = make_in_maps(query, key, value)
    res = run_bass_kernel_spmd(nc, in_maps, core_ids=list(range(NCORES)), **spmd_kwargs)
    return assemble_out(res.results), res


def kernel(query, key, value):
    out, _ = run(query, key, value)
    return out



# revision 7
# speedup vs baseline: 2.3974x; 2.3974x over previous
"""Linear attention (non-causal, elu+1 feature map) on 8 Trainium2 cores — v3.

Math per (batch b, head h), phi(x) = elu(x)+1:
    C_aug = phi(K)^T @ [V | 1]        # (64, 65): context + k_sum col
    numer = phi(Q) @ C_aug[:, :64]
    denom = phi(Q) @ C_aug[:, 64]
    out   = numer / denom             # eps=1e-6 negligible vs denom ~1e5

Key choices vs the fp32 baseline (233us):
  * fp16 inputs (host casts): PE matmuls at 1 cycle/row instead of 4, one
    LDWEIGHTS pass instead of two, half the HBM traffic (33MB -> 16.3MB per
    core; DMA roofline ~46-50us).
  * Both heads fused per matmul. Host packs [K0|K1|V0|1|V1|1] (258 cols per
    t-row) so mm1's stationary (128 K-cols) and moving (130 V-cols) APs are
    single-stride; psum diag blocks give C0_aug/C1_aug. mm2 streams a
    block-diagonal 128x130 C against contiguous 128-col phiQ chunks.
  * Output is one interleaved (128, j, h, e) bf16 tile per batch (host
    un-interleaves); bf16 never goes subnormal at our magnitudes, fp16 would.
  * Engine split (TRN2 Pool does no tensor arithmetic and can't touch PSUM,
    so Pool only memsets; DVE owns every two-tensor op):
      - phi(Q): min on DVE, exp on Act, (x+1)-max combine on DVE
      - phi(K): t=relu(-x), t=exp(-t) both on Act, combine on DVE
      - normalize: reciprocal (DVE) + one stride-0-broadcast
        scalar_tensor_tensor per 2-chunk psum group (DVE)
      - C diag-block copies from psum: one on DVE, one on Act
  * psum accumulation stays fp32.

Accuracy: fp16 quantization of K,V gives C entries ~0.03% rms error; through
the normalizer this lands ~5e-6 absolute on outputs vs the 2e-2 * 1e-3
per-element floor — ~100x margin (bf16 inputs would be ~3e-5, too close).
"""

from contextlib import ExitStack

import numpy as np

import concourse.bacc as bacc
import concourse.bass as bass
import concourse.mybir as mybir
import concourse.tile as tile
from concourse.bass_utils import run_bass_kernel_spmd

B = 4
T = 4096
D = 1024
H = 16
E = 64
EA = E + 1
NCORES = 8
HPC = H // NCORES  # 2 heads per core
KC = HPC * E  # 128 packed K columns per t-row
W2 = KC + HPC * EA  # 258 cols per kva row: [K0|K1|V0|1|V1|1]
P = 128
NT = T // P  # 32 t-tiles for mm1 (t = p*32 + n)
NJ = T // P  # 32 t-chunks for mm2 (t = 128*j + p)
GRP = 2  # mm2 chunks per psum tile (2*130*4B = 1040B <= 2KB bank)
F16 = mybir.dt.float16
F32 = mybir.dt.float32
BF16 = mybir.dt.bfloat16
AF = mybir.ActivationFunctionType
ALU = mybir.AluOpType


def build_nc():
    nc = bacc.Bacc("TRN2", target_bir_lowering=False, debug=False)
    qt = nc.dram_tensor("qt", [B, P, T], F16, kind="ExternalInput").ap()
    kva = nc.dram_tensor("kva", [B, T, W2], F16, kind="ExternalInput").ap()
    o = nc.dram_tensor("o", [B, P, NJ * HPC * E], BF16, kind="ExternalOutput").ap()

    with tile.TileContext(nc) as tc, ExitStack() as ctx:
        qt_pool = ctx.enter_context(tc.tile_pool(name="qt", bufs=2))
        kv_pool = ctx.enter_context(tc.tile_pool(name="kv", bufs=2))
        tmp_pool = ctx.enter_context(tc.tile_pool(name="tmp", bufs=4))
        c_pool = ctx.enter_context(tc.tile_pool(name="c", bufs=2))
        out_pool = ctx.enter_context(tc.tile_pool(name="out", bufs=2))
        r_pool = ctx.enter_context(tc.tile_pool(name="r", bufs=8))
        psc_pool = ctx.enter_context(tc.tile_pool(name="psc", bufs=2, space="PSUM"))
        pso_pool = ctx.enter_context(tc.tile_pool(name="pso", bufs=6, space="PSUM"))

        HW = NT * W2  # 8256 elems per partition

        for b in range(B):
            # ---- Q^T load + phi (both heads stacked on partitions) ----
            qt_t = qt_pool.tile([P, T], F16)
            for half in range(2):
                sl = slice(half * (T // 2), (half + 1) * (T // 2))
                nc.sync.dma_start(qt_t[:, sl], qt[b, :, sl])
                x = qt_t[:, sl]
                tq = tmp_pool.tile([P, T // 2], F16)
                nc.vector.tensor_scalar_min(tq[:], x, 0.0)
                nc.scalar.activation(tq[:], tq[:], AF.Exp)
                nc.vector.scalar_tensor_tensor(x, x, 1.0, tq[:], ALU.add, ALU.max)

            # ---- [K0|K1|V0|1|V1|1] load + phi(K) + mm1 (fused heads) ----
            kv = kv_pool.tile([P, HW], F16)
            kvr = kv[:].rearrange("p (n c) -> p n c", c=W2)
            psc = psc_pool.tile([P, HPC * EA], F32)
            for half in range(2):
                nsl = slice(half * (NT // 2), (half + 1) * (NT // 2))
                csl = slice(half * (HW // 2), (half + 1) * (HW // 2))
                nc.sync.dma_start(
                    kv[:, csl],
                    kva[b].rearrange("(p n) c -> p (n c)", p=P)[:, csl],
                )
                kview = kvr[:, nsl, 0:KC]
                tk = tmp_pool.tile([P, (NT // 2) * KC], F16)
                tk3 = tk[:].rearrange("p (n c) -> p n c", c=KC)
                # t = relu(-x) = -min(x,0); exp(-t) = exp(min(x,0))
                nc.scalar.activation(tk3, kview, AF.Relu, scale=-1.0)
                nc.scalar.activation(tk3, tk3, AF.Exp, scale=-1.0)
                nc.vector.scalar_tensor_tensor(
                    kview, kview, 1.0, tk3, ALU.add, ALU.max
                )
                for n in range(half * (NT // 2), (half + 1) * (NT // 2)):
                    nc.tensor.matmul(
                        psc[:],
                        lhsT=kvr[:, n, 0:KC],
                        rhs=kvr[:, n, KC:W2],
                        start=(n == 0),
                        stop=(n == NT - 1),
                    )

            # ---- block-diagonal C for the fused mm2 ----
            c_sb = c_pool.tile([P, HPC * EA], F16)
            nc.gpsimd.memset(c_sb[:], 0.0)
            nc.vector.tensor_copy(c_sb[0:E, 0:EA], psc[0:E, 0:EA])
            nc.scalar.copy(c_sb[E:P, EA : 2 * EA], psc[E:P, EA : 2 * EA])

            # ---- mm2 + normalize: out[t, (j,h,e)] over chunks of 128 t's ----
            ob = out_pool.tile([P, NJ * HPC * E], BF16)
            for g in range(NJ // GRP):
                ps = pso_pool.tile([P, GRP * HPC * EA], F32)
                for k in range(GRP):
                    j = g * GRP + k
                    nc.tensor.matmul(
                        ps[:, k * HPC * EA : (k + 1) * HPC * EA],
                        lhsT=qt_t[:, j * P : (j + 1) * P],
                        rhs=c_sb[:],
                        start=True,
                        stop=True,
                    )
                r = r_pool.tile([P, GRP * HPC], F32)
                nc.vector.reciprocal(r[:], ps[:, E::EA])
                # out = numer * (1/denom), denom broadcast along e via a
                # stride-0 AP
                numer = ps[:].rearrange("p (k h c) -> p k h c", k=GRP, h=HPC)[
                    :, :, :, 0:E
                ]
                rb = r[:].rearrange("p (k h c) -> p k h c", k=GRP, h=HPC)
                numer_b, rb = bass.broadcast_tensor_aps(numer, rb)
                oview = ob[
                    :, g * GRP * HPC * E : (g + 1) * GRP * HPC * E
                ].rearrange("p (k h c) -> p k h c", k=GRP, h=HPC)
                nc.vector.scalar_tensor_tensor(
                    oview, numer_b, 1.0, rb, ALU.mult, ALU.mult
                )
            nc.sync.dma_start(o[b], ob[:])
    nc.finalize()
    return nc


_NC_CACHE = None


def _get_nc():
    global _NC_CACHE
    if _NC_CACHE is None:
        _NC_CACHE = build_nc()
    return _NC_CACHE


def make_in_maps(query, key, value):
    query = np.asarray(query, dtype=np.float32)
    key = np.asarray(key, dtype=np.float32)
    value = np.asarray(value, dtype=np.float32)
    in_maps = []
    for c in range(NCORES):
        lo = c * HPC * E
        hi = lo + HPC * E
        qt = np.ascontiguousarray(
            query[:, :, lo:hi].transpose(0, 2, 1), dtype=np.float16
        )
        kva = np.empty((B, T, W2), np.float16)
        kva[..., 0:KC] = key[:, :, lo:hi]
        kva[..., KC : KC + E] = value[:, :, lo : lo + E]
        kva[..., KC + E] = 1.0
        kva[..., KC + EA : KC + EA + E] = value[:, :, lo + E : hi]
        kva[..., KC + EA + E] = 1.0
        in_maps.append({"qt": qt, "kva": kva})
    return in_maps


def assemble_out(results):
    out = np.empty((B, T, D), np.float32)
    for c in range(NCORES):
        # o[b, p, ((j*2 + h)*64 + e)] = out[b, t=128j+p, c*128 + h*64 + e]
        oc = np.asarray(results[c]["o"], dtype=np.float32)
        oc = oc.reshape(B, P, NJ, HPC, E).transpose(0, 2, 1, 3, 4)
        out[:, :, c * HPC * E : (c + 1) * HPC * E] = oc.reshape(B, T, HPC * E)
    return out


def run(query, key, value, **spmd_kwargs):
    nc = _get_nc()
    in_maps = make_in_maps(query, key, value)
    res = run_bass_kernel_spmd(nc, in_maps, core_ids=list(range(NCORES)), **spmd_kwargs)
    return assemble_out(res.results), res


def kernel(query, key, value):
    out, _ = run(query, key, value)
    return out


# revision 8
# speedup vs baseline: 2.7739x; 1.1571x over previous
"""Linear attention (non-causal, elu+1 feature map) on 8 Trainium2 cores — v5.

Math per (batch b, head h), phi(x) = elu(x)+1:
    C_aug = phi(K)^T @ [V | 1]        # (64, 65): context + k_sum col
    numer = phi(Q) @ C_aug[:, :64]
    denom = phi(Q) @ C_aug[:, 64]
    out   = numer / denom             # eps=1e-6 negligible vs denom ~1e5

Key choices vs the fp32 baseline (233us):
  * fp16 inputs (host casts): PE matmuls at 1 cycle/row instead of 4, one
    LDWEIGHTS pass instead of two, half the HBM traffic (33MB -> 16.3MB per
    core; DMA roofline ~46-50us).
  * Both heads fused per matmul. Host packs [K0|K1|V0|1|V1|1] (258 cols per
    t-row) so mm1's stationary (128 K-cols) and moving (130 V-cols) APs are
    single-stride; psum diag blocks give C0_aug/C1_aug. mm2 streams a
    block-diagonal 128x130 C against contiguous 128-col phiQ chunks.
  * phi is never materialized: phi(x) = exp(min(x,0)) + relu(x), and the PE
    adds the two halves by accumulating two matmuls into the same psum
    (scalar_tensor_tensor has no DVE fast mode — 1.08ns/elem — while
    min/max tensor_scalar runs at 0.31ns/elem and the extra matmul pass is
    ~60ns; measured on-HW in the v4 trace).
  * mm2 lhsT chunks are contiguous (t = 128j + p); output is one interleaved
    (p, j, h, e) bf16 tile per batch, host un-permutes. bf16 output never
    goes subnormal at our magnitudes (fp16 would below 6e-5).
  * Engine split (TRN2 Pool does no tensor arithmetic): DVE does min/relu
    (4x fp16 mode) + reciprocal + the normalize multiply (one stride-0
    broadcast scalar_tensor_tensor per 3-chunk psum group); Act does the two
    exp passes and the C diag-block casts; Pool only memsets.
  * psum accumulation stays fp32.

Accuracy: fp16 quantization of phi(K),V gives C entries ~0.2% rms error;
through the normalizer this lands ~1e-4 absolute worst-case on outputs vs
the 2e-2 per-element gate with its 1e-3 floor (measured 1.5e-2 max rel).
"""

from contextlib import ExitStack

import numpy as np

import concourse.bacc as bacc
import concourse.bass as bass
import concourse.mybir as mybir
import concourse.tile as tile
from concourse.bass_utils import run_bass_kernel_spmd

B = 4
T = 4096
D = 1024
H = 16
E = 64
EA = E + 1
NCORES = 8
HPC = H // NCORES  # 2 heads per core
KC = HPC * E  # 128 packed K columns per t-row
W2 = KC + HPC * EA  # 258 cols per kva row: [K0|K1|V0|1|V1|1]
P = 128
NT = T // P  # 32 t-tiles for mm1 (t = p*32 + n)
NJ = T // P  # 32 t-chunks for mm2 (t = 128*j + p)
F16 = mybir.dt.float16
F32 = mybir.dt.float32
BF16 = mybir.dt.bfloat16
AF = mybir.ActivationFunctionType
ALU = mybir.AluOpType

# mm2 psum grouping: chunks per tile (3*130*4B = 1560B <= 2KB bank)
GRPS = [3, 3, 3, 3, 3, 3, 3, 3, 3, 3, 2]
assert sum(GRPS) == NJ


def build_nc():
    nc = bacc.Bacc("TRN2", target_bir_lowering=False, debug=False)
    qt = nc.dram_tensor("qt", [B, P, T], F16, kind="ExternalInput").ap()
    kva = nc.dram_tensor("kva", [B, T, W2], F16, kind="ExternalInput").ap()
    o = nc.dram_tensor("o", [B, P, NJ * HPC * E], BF16, kind="ExternalOutput").ap()

    with tile.TileContext(nc) as tc, ExitStack() as ctx:
        qt_pool = ctx.enter_context(tc.tile_pool(name="qt", bufs=2))
        kv_pool = ctx.enter_context(tc.tile_pool(name="kv", bufs=2))
        tmp_pool = ctx.enter_context(tc.tile_pool(name="tmp", bufs=8))
        c_pool = ctx.enter_context(tc.tile_pool(name="c", bufs=2))
        out_pool = ctx.enter_context(tc.tile_pool(name="out", bufs=2))
        r_pool = ctx.enter_context(tc.tile_pool(name="r", bufs=8))
        psc_pool = ctx.enter_context(tc.tile_pool(name="psc", bufs=2, space="PSUM"))
        pso_pool = ctx.enter_context(tc.tile_pool(name="pso", bufs=6, space="PSUM"))

        HW = NT * W2  # 8256 elems per partition
        TH = T // 2  # 2048 cols per half

        for b in range(B):
            # ---- Q^T load; E_q = exp(min(q,0)) in tmp, R_q = relu(q) ----
            qt_t = qt_pool.tile([P, T], F16)
            eq = []
            for half in range(2):
                sl = slice(half * TH, (half + 1) * TH)
                nc.sync.dma_start(qt_t[:, sl], qt[b, :, sl])
                x = qt_t[:, sl]
                tq = tmp_pool.tile([P, TH], F16)
                nc.vector.tensor_scalar_min(tq[:], x, 0.0)
                nc.scalar.activation(tq[:], tq[:], AF.Exp)
                nc.vector.tensor_scalar_max(x, x, 0.0)
                eq.append(tq)

            # ---- [K0|K1|V0|1|V1|1] load; E_k in tmp, R_k in place; mm1 ----
            kv = kv_pool.tile([P, HW], F16)
            kvr = kv[:].rearrange("p (n c) -> p n c", c=W2)
            psc = psc_pool.tile([P, HPC * EA], F32)
            NH = NT // 2  # 16 n-tiles per half
            for half in range(2):
                nsl = slice(half * NH, (half + 1) * NH)
                csl = slice(half * (HW // 2), (half + 1) * (HW // 2))
                nc.sync.dma_start(
                    kv[:, csl],
                    kva[b].rearrange("(p n) c -> p (n c)", p=P)[:, csl],
                )
                kview = kvr[:, nsl, 0:KC]
                tk = tmp_pool.tile([P, NH * KC], F16)
                tk3 = tk[:].rearrange("p (n c) -> p n c", c=KC)
                nc.vector.tensor_scalar_min(tk3, kview, 0.0)
                nc.scalar.activation(tk3, tk3, AF.Exp)
                nc.vector.tensor_scalar_max(kview, kview, 0.0)
                for n in range(half * NH, (half + 1) * NH):
                    vaug = kvr[:, n, KC:W2]
                    nc.tensor.matmul(
                        psc[:],
                        lhsT=tk[:, (n - half * NH) * KC : (n - half * NH + 1) * KC],
                        rhs=vaug,
                        start=(n == 0),
                        stop=False,
                    )
                    nc.tensor.matmul(
                        psc[:],
                        lhsT=kvr[:, n, 0:KC],
                        rhs=vaug,
                        start=False,
                        stop=(n == NT - 1),
                    )

            # ---- block-diagonal C for the fused mm2 ----
            c_sb = c_pool.tile([P, HPC * EA], F16)
            nc.gpsimd.memset(c_sb[:], 0.0)
            nc.scalar.copy(c_sb[0:E, 0:EA], psc[0:E, 0:EA])
            nc.scalar.copy(c_sb[E:P, EA : 2 * EA], psc[E:P, EA : 2 * EA])

            # ---- mm2 (E and R accumulated) + normalize ----
            ob = out_pool.tile([P, NJ * HPC * E], BF16)
            j = 0
            for grp in GRPS:
                ps = pso_pool.tile([P, grp * HPC * EA], F32)
                for k in range(grp):
                    jj = j + k
                    half, jh = jj // 16, jj % 16
                    blk = ps[:, k * HPC * EA : (k + 1) * HPC * EA]
                    nc.tensor.matmul(
                        blk,
                        lhsT=eq[half][:, jh * P : (jh + 1) * P],
                        rhs=c_sb[:],
                        start=True,
                        stop=False,
                    )
                    nc.tensor.matmul(
                        blk,
                        lhsT=qt_t[:, jj * P : (jj + 1) * P],
                        rhs=c_sb[:],
                        start=False,
                        stop=True,
                    )
                r = r_pool.tile([P, grp * HPC], F32)
                nc.vector.reciprocal(r[:], ps[:, E::EA])
                numer = ps[:].rearrange("p (k h c) -> p k h c", k=grp, h=HPC)[
                    :, :, :, 0:E
                ]
                rb = r[:].rearrange("p (k h c) -> p k h c", k=grp, h=HPC)
                numer_b, rb = bass.broadcast_tensor_aps(numer, rb)
                oview = ob[
                    :, j * HPC * E : (j + grp) * HPC * E
                ].rearrange("p (k h c) -> p k h c", k=grp, h=HPC)
                nc.vector.scalar_tensor_tensor(
                    oview, numer_b, 1.0, rb, ALU.mult, ALU.mult
                )
                j += grp
            nc.sync.dma_start(o[b], ob[:])
    nc.finalize()
    return nc


_NC_CACHE = None


def _get_nc():
    global _NC_CACHE
    if _NC_CACHE is None:
        _NC_CACHE = build_nc()
    return _NC_CACHE


def make_in_maps(query, key, value):
    query = np.asarray(query, dtype=np.float32)
    key = np.asarray(key, dtype=np.float32)
    value = np.asarray(value, dtype=np.float32)
    in_maps = []
    for c in range(NCORES):
        lo = c * HPC * E
        hi = lo + HPC * E
        qt = np.ascontiguousarray(
            query[:, :, lo:hi].transpose(0, 2, 1), dtype=np.float16
        )
        kva = np.empty((B, T, W2), np.float16)
        kva[..., 0:KC] = key[:, :, lo:hi]
        kva[..., KC : KC + E] = value[:, :, lo : lo + E]
        kva[..., KC + E] = 1.0
        kva[..., KC + EA : KC + EA + E] = value[:, :, lo + E : hi]
        kva[..., KC + EA + E] = 1.0
        in_maps.append({"qt": qt, "kva": kva})
    return in_maps


def assemble_out(results):
    out = np.empty((B, T, D), np.float32)
    for c in range(NCORES):
        # o[b, p, ((j*2 + h)*64 + e)] = out[b, t=128j+p, c*128 + h*64 + e]
        oc = np.asarray(results[c]["o"], dtype=np.float32)
        oc = oc.reshape(B, P, NJ, HPC, E).transpose(0, 2, 1, 3, 4)
        out[:, :, c * HPC * E : (c + 1) * HPC * E] = oc.reshape(B, T, HPC * E)
    return out


def run(query, key, value, **spmd_kwargs):
    nc = _get_nc()
    in_maps = make_in_maps(query, key, value)
    res = run_bass_kernel_spmd(nc, in_maps, core_ids=list(range(NCORES)), **spmd_kwargs)
    return assemble_out(res.results), res


def kernel(query, key, value):
    out, _ = run(query, key, value)
    return out
